# revision 1
# baseline (speedup 1.0000x reference)
"""Trainium2 Bass kernel for nn_Classifier_64587718197982 (spiking CNN).

Network (per reference):
  3x [conv3x3(C=128, pad=1, no bias) -> TDBN (batchnorm over T,B,H,W) -> LIF]
  -> mean over (H,W) -> mean over T -> FC(128->10)

Sharding: data-parallel over batch B=32 across 8 NeuronCores (4 images/core).
TDBN mean/var become a tiny [128,2] AllReduce per layer (~100us each, the
dominant fixed cost - measured via the K_NOAR probe).

Design (fully SBUF-resident activations, cross-layer pipelining):
  - y (conv output) lives in ONE SBUF buffer [C, 32, 1024] f32, overwritten
    in place by the next layer's conv: drain(l+1, img) transitively depends
    on LIF_l(img)'s read of y[img] via the spike, so WAR order is free.
    f32 is required: fp16 y on the LIF path alone costs ~1.5e-2 rel err
    (spike flips near threshold; see quant_study.py).
  - Spikes rotate through a small 8-tile pool (each spike image has exactly
    one consumer - the next conv - since LIF gates are computed from u).
  - LIF in u-space (u = mem/s): u_t = (y_t + d') + gate*u_{t-1},
    gate = 0.25*(u_{t-1} <= th'), spike = (u_t > th'), th' = 0.5/s,
    d' = d/s.  No per-image affine pass.  Post-AllReduce algebra is 8 ops
    (reciprocal-of-bnw terms precomputed off the critical path).
  - Engine partition (avoids in-order queue head-blocking cycles):
      DVE (vector): all LIF ops (gate, u*gate, u update, spike) - pre-conv
      PE: conv matmuls (9 shifted range-restricted MMs per psum half)
      Act (scalar): PSUM->SBUF drains (accum_out = sum(y) free) plus a
        Square pass (accum_out = sum(y^2)); L3 Sign+accum pooling
      Pool (gpsimd): collectives, head, part of the L3 tail multiplies
    BN stats therefore cost zero DVE time and are Act-local (drain ->
    square same-engine-serial), so they cannot head-block the LIF stream.
    The AllReduce payload is raw [sum(y), sum(y^2)].
    Warm-up matmuls at kernel start absorb the weight-DMA waits that
    walrus otherwise rejects on split-out standalone Ldweights.
    Emission: for t,b: LIF_l(t,b) then conv_{l+1}(t,b), so matmuls hide
    all vector work; only conv1 (no LIF before it) and LIF3+head are bare.
  - Layer1 numerics: main term w16*x16 in fp16 + BOTH correction terms
    (wlo*xhi + whi*xlo) packed into one fp8-e5m2 DoubleRow matmul per tap
    (two contraction halves per MM), scale-balanced by 2^+-6 so operands
    stay in e5m2 normal range and products land at true scale in PSUM.
    Corrections are ~2^-11 of the main term, so 2 mantissa bits suffice.
    Layers 2/3: single fp16 term (spikes are exact in fp16; single-term
    e4m3 measured 2.8e-2 rel err - fails the gate, see quant_study2.py).
"""
import numpy as np
import ml_dtypes
from contextlib import ExitStack

import concourse.bass as bass
import concourse.mybir as mybir
import concourse.tile as tile
from concourse import bass_isa
from concourse import bacc
from concourse.bass_utils import run_bass_kernel_spmd

F32 = mybir.dt.float32
FP16 = mybir.dt.float16
FP8E5 = mybir.dt.float8e5
AF = mybir.ActivationFunctionType
ALU = mybir.AluOpType

T, B, C, H, W = 8, 32, 128, 32, 32
NCORES = 8
BL = B // NCORES          # images per core per timestep
HW = H * W                # 1024
NIMG = T * BL             # 32 images per core
DECAY = 0.25
THRESH = 0.5
BN_EPS = 1e-5
POOL_N = float(T * HW)    # pooling divisor 8192
import os as _os
SIM1 = _os.environ.get("K_SIM1", "") == "1"  # single-core sim (no collectives)
NOAR = _os.environ.get("K_NOAR", "") == "1"  # timing probe: skip AllReduce (wrong results)

# layer-1 term scheme: "dr" = fp16 main + e5m2 DoubleRow corrections;
# integers fall back to the fp16 multi-term schemes (1/2/3)
L1_MODE = _os.environ.get("K_L1", "dr")
TERMS_L1 = 0 if L1_MODE == "dr" else int(L1_MODE)
# e5m2 correction-term balance scale (wlo*S paired with x/S; whi/S with xlo*S)
DR_S = 64.0
# psum tile columns (512 = half image per bank, 1024 = full image, 2 banks)
PSUM_COLS = int(_os.environ.get("K_PSUM_COLS", "512"))

# conv shift order: (1,1) first so the start=True matmul covers the full tile
SHIFTS = [(1, 1)] + [(dy, dx) for dy in range(3) for dx in range(3)
                     if not (dy == 1 and dx == 1)]


def _conv_image(nc, psum_pool, y_img, terms, dr_terms=(), acc=None):
    """Conv for one image.  terms = [(w_sb[C,9,C], x3d[C,H,W]), ...];
    dr_terms = [(w8[C,9,2,C], x8[C,2,H,W]), ...] fp8 DoubleRow pairs (two
    contraction halves per matmul).  Drains PSUM (Act) into y_img.
    bn_stats is emitted separately at window end (DVE, after all LIF ops)
    to avoid queue head-blocking.

    PSUM_COLS=1024 uses one 2-bank PSUM tile per image: fp16 matmuls run
    full-height (9 instead of 18 per image), DR matmuls stay per-half
    (fp8 moving operand is capped at 1024 elements = 512 output cols)."""
    nhalf = HW // PSUM_COLS
    rh = H // nhalf
    for h in range(nhalf):
        r_base = h * rh
        pt = psum_pool.tile([C, rh * W], F32, tag="pt")
        p3 = pt.rearrange("c (r w) -> c r w", r=rh)
        n_dr = len(SHIFTS) * len(dr_terms) * (rh // 16)
        n_mm = len(SHIFTS) * len(terms) + n_dr
        i_mm = 0
        for (dy, dx) in SHIFTS:
            oy, ox = dy - 1, dx - 1
            r0 = max(r_base, -oy)
            r1 = min(r_base + rh, H - oy)
            c0 = max(0, -ox)
            c1 = min(W, W - ox)
            k = dy * 3 + dx
            for (w_sb, x3d) in terms:
                nc.tensor.matmul(
                    p3[:, r0 - r_base:r1 - r_base, c0:c1],
                    w_sb[:, k, :],
                    x3d[:, r0 + oy:r1 + oy, c0 + ox:c1 + ox],
                    start=(i_mm == 0), stop=(i_mm == n_mm - 1),
                )
                i_mm += 1
            for (w8, x8) in dr_terms:
                for q0 in range(r_base, r_base + rh, 16):
                    q1 = min(q0 + 16, r_base + rh)
                    s0 = max(q0, r0)
                    s1 = min(q1, r1)
                    if s1 <= s0:
                        i_mm += 1
                        continue
                    nc.tensor.matmul(
                        p3[:, s0 - r_base:s1 - r_base, c0:c1],
                        w8[:, k],
                        x8[:, :, s0 + oy:s1 + oy, c0 + ox:c1 + ox],
                        start=(i_mm == 0), stop=(i_mm == n_mm - 1),
                        perf_mode=mybir.MatmulPerfMode.DoubleRow,
                    )
                    i_mm += 1
        # drain on Act (post-conv engine); accum_out gives sum(y) free.
        # A second Act pass (Square) accumulates sum(y^2): both read/write
        # Act-local data, so stats cost zero DVE time and cannot
        # head-block the LIF stream.
        y_half = y_img[:, h * rh * W:(h + 1) * rh * W]
        if acc is None or _os.environ.get("K_NOACC") == "1":
            nc.scalar.copy(out=y_half, in_=pt)
        else:
            sum_slot, sq_slot, scratch_pool, slot = acc
            nc.scalar.activation(y_half, pt, AF.Copy,
                                 accum_out=sum_slot[:, slot + h:slot + h + 1])
            scr = scratch_pool.tile([C, rh * W], FP16, tag="sqscr")
            nc.scalar.activation(scr, y_half, AF.Square,
                                 accum_out=sq_slot[:, slot + h:slot + h + 1])


def _layer_stats(nc, sb1, stats_buf, pre, eps_t, cc_in, cc_out, lname):
    """bn_aggr + AllReduce + u-space coefficient computation.

    Returns (dp, thp, negthp): d' = d/s, th' = THRESH/s, -th'."""
    sums, sqs = stats_buf
    # cc payload: raw [sum(y), sum(y^2)] per channel (reduced over slots)
    cc = sb1.tile([C, 2], F32, tag=f"cc{lname}")
    nc.vector.tensor_reduce(cc[:, 0:1], sums,
                            axis=mybir.AxisListType.X, op=ALU.add)
    nc.vector.tensor_reduce(cc[:, 1:2], sqs,
                            axis=mybir.AxisListType.X, op=ALU.add)
    nc.sync.dma_start(out=cc_in[:, :], in_=cc)
    if not SIM1 and not NOAR:
        nc.gpsimd.collective_compute(
            "AllReduce", ALU.add,
            replica_groups=[list(range(NCORES))],
            ins=[cc_in[:, :]], outs=[cc_out[:, :]],
        )
    ccr = sb1.tile([C, 2], F32, tag=f"ccr{lname}")
    nc.sync.dma_start(out=ccr, in_=cc_in[:, :] if (SIM1 or NOAR) else cc_out[:, :])
    inv_n = 1.0 / ((1 if SIM1 else NCORES) * NIMG * HW)
    # short critical-path algebra: th' = 0.5*sd/bnw, d' = bnb*sd/bnw - mean
    # (rbw = 0.5/bnw etc. precomputed off the critical path)
    rbw, nrbw, bbrb = pre
    mean = sb1.tile([C, 1], F32, tag=f"mean{lname}")
    nc.vector.tensor_scalar(mean, ccr[:, 0:1], inv_n, None, op0=ALU.mult)
    msq = sb1.tile([C, 1], F32, tag=f"msq{lname}")
    nc.vector.tensor_tensor(msq, mean, mean, op=ALU.mult)
    var = sb1.tile([C, 1], F32, tag=f"var{lname}")
    nc.vector.scalar_tensor_tensor(var, ccr[:, 1:2], inv_n, msq,
                                   op0=ALU.mult, op1=ALU.subtract)
    sd = sb1.tile([C, 1], F32, tag=f"sd{lname}")
    nc.scalar.activation(sd, var, AF.Sqrt, bias=eps_t, scale=1.0)
    thp = sb1.tile([C, 1], F32, tag=f"thp{lname}")
    nc.vector.tensor_tensor(thp, sd, rbw, op=ALU.mult)
    negthp = sb1.tile([C, 1], F32, tag=f"negthp{lname}")
    nc.vector.tensor_tensor(negthp, sd, nrbw, op=ALU.mult)
    dp = sb1.tile([C, 1], F32, tag=f"dp{lname}")
    nc.vector.scalar_tensor_tensor(dp, sd, bbrb, mean,
                                   op0=ALU.mult, op1=ALU.subtract)
    return dp, thp, negthp


def build():
    nc = bacc.Bacc("TRN2", target_bir_lowering=False, debug=False,
                   num_devices=1 if SIM1 else NCORES)

    # --- I/O ---
    xhi_d = nc.dram_tensor("xhi", [T, BL, C, HW], FP16, kind="ExternalInput")
    xlo_d = xc8_d = w1c8_d = None
    if L1_MODE == "dr":
        xc8_d = nc.dram_tensor("xc8", [T, BL, C, 2, HW], FP8E5,
                               kind="ExternalInput")
        w1c8_d = nc.dram_tensor("w1c8", [C, 9, 2, C], FP8E5,
                                kind="ExternalInput")
    elif TERMS_L1 >= 2:
        xlo_d = nc.dram_tensor("xlo", [T, BL, C, HW], FP16,
                               kind="ExternalInput")
    w_d = {}
    for l in (1, 2, 3):
        w_d[(l, "hi")] = nc.dram_tensor(f"w{l}hi", [C, 9, C], FP16,
                                        kind="ExternalInput")
    if TERMS_L1 >= 3:
        w_d[(1, "lo")] = nc.dram_tensor("w1lo", [C, 9, C], FP16,
                                        kind="ExternalInput")
    bn_d = {}
    for l in (1, 2, 3):
        bn_d[(l, "w")] = nc.dram_tensor(f"bnw{l}", [C, 1], F32,
                                        kind="ExternalInput")
        bn_d[(l, "b")] = nc.dram_tensor(f"bnb{l}", [C, 1], F32,
                                        kind="ExternalInput")
    fcw_d = nc.dram_tensor("fcw", [C, 10], F32, kind="ExternalInput")
    fcb_d = nc.dram_tensor("fcb", [1, 10], F32, kind="ExternalInput")
    out_d = nc.dram_tensor("out", [1, BL * 10], F32, kind="ExternalOutput")

    # --- internal DRAM (collective buffers only) ---
    cc_bufs = {}
    for l in (1, 2, 3):
        cc_bufs[l] = (
            nc.dram_tensor(f"cc_in{l}", [C, 2], F32),
            nc.dram_tensor(f"cc_out{l}", [C, 2], F32, addr_space="Shared"),
        )

    with ExitStack() as ctx:
        tc = ctx.enter_context(tile.TileContext(nc))
        sb1 = ctx.enter_context(tc.tile_pool(name="sb1", bufs=1))
        xpool = ctx.enter_context(tc.tile_pool(name="xpool", bufs=4))
        gate_pool = ctx.enter_context(tc.tile_pool(name="gate", bufs=3))
        stage_pool = ctx.enter_context(tc.tile_pool(name="stage", bufs=3))
        mem_pool = ctx.enter_context(tc.tile_pool(name="mem", bufs=1))
        psum_pool = ctx.enter_context(
            tc.tile_pool(name="psum", bufs=(8 * 512) // PSUM_COLS,
                         space="PSUM"))

        # --- load layer-1 weights only; everything else streams during
        # the conv1 window so the first x DMA isn't queued behind ~1.2MB
        # of constants ---
        w_sb = {}
        for key in ((1, "hi"),) + (((1, "lo"),) if (1, "lo") in w_d else ()):
            w_sb[key] = sb1.tile([C, 9, C], FP16,
                                 name=f"w{key[0]}{key[1]}",
                                 tag=f"w{key[0]}{key[1]}")
            nc.sync.dma_start(out=w_sb[key], in_=w_d[key][:, :, :])
        w1c8_sb = None
        if L1_MODE == "dr":
            w1c8_sb = sb1.tile([C, 9, 2, C], FP8E5, name="w1c8", tag="w1c8")
            nc.sync.dma_start(out=w1c8_sb, in_=w1c8_d[:, :, :, :])

        # Warm-up matmuls: touch each weight tensor once with a fused
        # (self-loading) matmul so the weight-DMA waits attach to a Matmult.
        # Without these, the first standalone Ldweights per weight carries
        # the DMA wait and walrus rejects it ("not compatible with LDW
        # optimization").
        def warmup_mm(w2d):
            pt = psum_pool.tile([C, PSUM_COLS], F32, tag="pt")
            nc.tensor.matmul(pt[:, 0:1], w2d, w2d[:, 0:1],
                             start=True, stop=True)

        warmup_mm(w_sb[(1, "hi")][:, 4, :])
        if L1_MODE == "dr":
            warmup_mm(w1c8_sb[:, 4, 0, :])

        # persistent SBUF activations: y in f32 (fp16 y on the LIF path
        # costs ~1.5e-2 rel err); spikes rotate through a small pool (each
        # spike image has exactly one consumer: the next conv)
        ybuf = sb1.tile([C, NIMG, HW], F32)        # conv out, all layers
        spk_pool = ctx.enter_context(tc.tile_pool(name="spk", bufs=8))

        # L3 pooled spike counts per (t, b)
        poolbuf = sb1.tile([C, BL, T], F32)

        stats = {}
        for l in (1, 2, 3):
            ysum = sb1.tile([C, NIMG * 2], F32, name=f"ysum{l}",
                            tag=f"sum{l}")
            ysq = sb1.tile([C, NIMG * 2], F32, name=f"ysq{l}",
                           tag=f"sq{l}")
            stats[l] = (ysum, ysq)
        scratch_pool = ctx.enter_context(tc.tile_pool(name="sqscr", bufs=2))

        # =============== layer 1 conv (standalone window) ===============
        for t in range(T):
            for b in range(BL):
                img = t * BL + b
                xhi = xpool.tile([C, HW], FP16, tag="xin")
                nc.sync.dma_start(out=xhi, in_=xhi_d[t, b, :, :])
                xhi3 = xhi.rearrange("c (h w) -> c h w", h=H)
                terms = [(w_sb[(1, "hi")], xhi3)]
                dr_terms = []
                if L1_MODE == "dr":
                    x8 = xpool.tile([C, 2, HW], FP8E5, tag="x8")
                    nc.sync.dma_start(out=x8, in_=xc8_d[t, b])
                    x84 = x8.rearrange("c two (h w) -> c two h w", h=H)
                    dr_terms.append((w1c8_sb, x84))
                else:
                    if TERMS_L1 >= 3:
                        terms.append((w_sb[(1, "lo")], xhi3))
                    if TERMS_L1 >= 2:
                        xlo = xpool.tile([C, HW], FP16, tag="xin")
                        nc.sync.dma_start(out=xlo, in_=xlo_d[t, b, :, :])
                        xlo3 = xlo.rearrange("c (h w) -> c h w", h=H)
                        terms.append((w_sb[(1, "hi")], xlo3))
                _conv_image(nc, psum_pool, ybuf[:, img], terms, dr_terms,
                            acc=(stats[1][0], stats[1][1], scratch_pool,
                                 2 * img))

        # deferred constants: stream in under the conv1 window
        for key in ((2, "hi"), (3, "hi")):
            w_sb[key] = sb1.tile([C, 9, C], FP16,
                                 name=f"w{key[0]}{key[1]}",
                                 tag=f"w{key[0]}{key[1]}")
            nc.sync.dma_start(out=w_sb[key], in_=w_d[key][:, :, :])
        bn_sb = {}
        for key, dt_ in bn_d.items():
            bn_sb[key] = sb1.tile([C, 1], F32, name=f"bn{key[1]}{key[0]}",
                                  tag=f"bn{key[1]}{key[0]}")
            nc.sync.dma_start(out=bn_sb[key], in_=dt_[:, :])
        fcw_sb = sb1.tile([C, 10], F32)
        nc.sync.dma_start(out=fcw_sb, in_=fcw_d[:, :])
        fcb_sb = sb1.tile([1, 10], F32)
        nc.sync.dma_start(out=fcb_sb, in_=fcb_d[:, :])
        eps_t = sb1.tile([C, 1], F32)
        nc.vector.memset(eps_t, BN_EPS)
        # off-critical-path per-layer constants: 0.5/bnw, -0.5/bnw, bnb/bnw
        pre = {}
        for l in (1, 2, 3):
            rb = sb1.tile([C, 1], F32, tag=f"rbw{l}")
            nc.vector.reciprocal(out=rb, in_=bn_sb[(l, "w")])
            rbw = sb1.tile([C, 1], F32, tag=f"rbwth{l}")
            nc.vector.tensor_scalar(rbw, rb, THRESH, None, op0=ALU.mult)
            nrbw = sb1.tile([C, 1], F32, tag=f"nrbw{l}")
            nc.vector.tensor_scalar(nrbw, rb, -THRESH, None, op0=ALU.mult)
            bbrb = sb1.tile([C, 1], F32, tag=f"bbrb{l}")
            nc.vector.tensor_tensor(bbrb, bn_sb[(l, "b")], rb, op=ALU.mult)
            pre[l] = (rbw, nrbw, bbrb)
        warmup_mm(w_sb[(2, "hi")][:, 4, :])
        warmup_mm(w_sb[(3, "hi")][:, 4, :])

        dp1, thp1, _ = _layer_stats(nc, sb1, stats[1], pre[1], eps_t,
                                    *cc_bufs[1], "l1")

        # ====== windows 1&2: LIF_l (DVE) interleaved with conv_{l+1} (PE) ===
        def lif_conv_window(dp, thp, wnext, stats_next):
            """Emit LIF for layer l -> spk, immediately followed per-image by
            conv of layer l+1 reading that spike.  u ops all on DVE."""
            u = mem_pool.tile([C, BL * HW], F32, tag="mem")
            u4 = u.rearrange("c (b p) -> c b p", b=BL)
            prev_spk = [None] * BL
            for t in range(T):
                for b in range(BL):
                    img = t * BL + b
                    yt = ybuf[:, img]
                    if t == 0:
                        nc.vector.tensor_scalar(u4[:, b], yt, dp, None,
                                                op0=ALU.add)
                    else:
                        # gate = 0.25*(1 - spike_prev): fp16 input, 2x DVE
                        gate = gate_pool.tile([C, HW], FP16, tag="gate")
                        nc.vector.tensor_scalar(gate, prev_spk[b],
                                                -DECAY, DECAY,
                                                op0=ALU.mult, op1=ALU.add)
                        nc.vector.tensor_tensor(u4[:, b], u4[:, b], gate,
                                                op=ALU.mult)
                        nc.vector.scalar_tensor_tensor(
                            u4[:, b], yt, dp, u4[:, b],
                            op0=ALU.add, op1=ALU.add)
                    spkt = spk_pool.tile([C, HW], FP16, tag="spk")
                    nc.vector.tensor_scalar(spkt, u4[:, b], thp, None,
                                            op0=ALU.is_gt)
                    prev_spk[b] = spkt
                    spk3 = spkt.rearrange("c (h w) -> c h w", h=H)
                    _conv_image(nc, psum_pool, ybuf[:, img],
                                [(wnext, spk3)],
                                acc=(stats_next[0], stats_next[1],
                                     scratch_pool, 2 * img))

        lif_conv_window(dp1, thp1, w_sb[(2, "hi")], stats[2])
        dp2, thp2, _ = _layer_stats(nc, sb1, stats[2], pre[2], eps_t,
                                    *cc_bufs[2], "l2")

        lif_conv_window(dp2, thp2, w_sb[(3, "hi")], stats[3])
        dp3, _, negthp3 = _layer_stats(nc, sb1, stats[3], pre[3], eps_t,
                                       *cc_bufs[3], "l3")

        # =============== layer 3 LIF (sign/accum trick) ===============
        u = mem_pool.tile([C, BL * HW], F32, tag="mem")
        u4 = u.rearrange("c (b p) -> c b p", b=BL)
        prev_sgn = [None] * BL
        for t in range(T):
            for b in range(BL):
                img = t * BL + b
                yt = ybuf[:, img]
                if t == 0:
                    nc.vector.tensor_scalar(u4[:, b], yt, dp3, None,
                                            op0=ALU.add)
                else:
                    # balance DVE-class work across DVE/GpSimd - the tail
                    # has no conv to hide behind (GpSimd runs ~2x slower,
                    # so it takes the gates+mults for 2 of 4 b-images
                    # while DVE keeps the DVE-only stt updates)
                    eng = nc.vector if b < 2 else nc.gpsimd
                    # gate from sign in {-1,1}: 0.125 - 0.125*sgn
                    gate = gate_pool.tile([C, HW], FP16, tag="gate")
                    eng.tensor_scalar(gate, prev_sgn[b],
                                      -DECAY / 2.0, DECAY / 2.0,
                                      op0=ALU.mult, op1=ALU.add)
                    eng.tensor_tensor(u4[:, b], u4[:, b], gate,
                                      op=ALU.mult)
                    # scalar_tensor_tensor with AP scalar is DVE-only
                    nc.vector.scalar_tensor_tensor(
                        u4[:, b], yt, dp3, u4[:, b],
                        op0=ALU.add, op1=ALU.add)
                st = stage_pool.tile([C, HW], FP16, tag="stage")
                nc.scalar.activation(st, u4[:, b], AF.Sign, bias=negthp3,
                                     scale=1.0,
                                     accum_out=poolbuf[:, b, t:t + 1])
                prev_sgn[b] = st

        # =============== head: pooling + FC ===============
        feat = sb1.tile([C, BL], F32)
        nc.vector.tensor_reduce(feat.unsqueeze(2), poolbuf,
                                axis=mybir.AxisListType.X, op=ALU.add)
        nc.vector.tensor_scalar(feat, feat, 0.5, T * HW / 2.0,
                                op0=ALU.mult, op1=ALU.add)
        # prod[c, b, k] = feat[c, b] * fcw[c, k]
        prod = sb1.tile([C, BL, 10], F32)
        nc.vector.tensor_tensor(
            prod, feat.unsqueeze(2).broadcast_to([C, BL, 10]),
            fcw_sb.unsqueeze(1).broadcast_to([C, BL, 10]), op=ALU.mult)
        red = sb1.tile([C, BL, 10], F32)
        nc.gpsimd.partition_all_reduce(red, prod, channels=C,
                                       reduce_op=bass_isa.ReduceOp.add)
        ofin = sb1.tile([1, BL, 10], F32)
        nc.vector.tensor_scalar(ofin, red[0:1], 1.0 / POOL_N, None,
                                op0=ALU.mult)
        nc.vector.tensor_tensor(
            ofin, ofin, fcb_sb.unsqueeze(1).broadcast_to([1, BL, 10]),
            op=ALU.add)
        nc.sync.dma_start(out=out_d[:, :],
                          in_=ofin.rearrange("c b k -> c (b k)"))

    # walrus rejects the standalone InstLdweights this pass splits out for
    # our multi-wait first-of-image matmuls ("not compatible with LDW
    # optimization"); excess waits lower to event-semaphore chains instead.
    nc.move_matmul_waits_to_ldweights = lambda: None
    nc.compile()
    return nc


_NC_CACHE = {}


def _get_nc():
    if "nc" not in _NC_CACHE:
        _NC_CACHE["nc"] = build()
    return _NC_CACHE["nc"]


def _hi_lo(a):
    hi = a.astype(np.float16)
    lo = (a - hi.astype(np.float32)).astype(np.float16)
    return hi, lo


def _e5m2(a, scale):
    return (np.asarray(a, np.float32) * scale).astype(ml_dtypes.float8_e5m2)


def make_in_maps(inp, conv_ws, bns, fc_w, fc_b):
    """Build the 8 per-core input maps from full (numpy) model inputs."""
    common = {}
    for li, w in enumerate(conv_ws, start=1):
        wt = np.ascontiguousarray(
            w.transpose(1, 2, 3, 0).reshape(C, 9, C))  # [I, k, O]
        hi, lo = _hi_lo(wt)
        common[f"w{li}hi"] = hi
        if li == 1 and L1_MODE == "dr":
            # pair 0: wlo*S (x main moves at x/S); pair 1: whi/S (xlo*S)
            w1c8 = np.empty((C, 9, 2, C), dtype=ml_dtypes.float8_e5m2)
            w1c8[:, :, 0, :] = _e5m2(lo.astype(np.float32), DR_S)
            w1c8[:, :, 1, :] = _e5m2(hi.astype(np.float32), 1.0 / DR_S)
            common["w1c8"] = w1c8
        if li == 1 and TERMS_L1 >= 3:
            common["w1lo"] = lo
        common[f"bnw{li}"] = np.ascontiguousarray(
            bns[li - 1][0].reshape(C, 1))
        common[f"bnb{li}"] = np.ascontiguousarray(
            bns[li - 1][1].reshape(C, 1))
    common["fcw"] = np.ascontiguousarray(fc_w.T)          # [C, 10]
    common["fcb"] = np.ascontiguousarray(fc_b.reshape(1, 10))

    in_maps = []
    for cid in range(NCORES):
        xc = np.ascontiguousarray(
            inp[:, cid * BL:(cid + 1) * BL].reshape(T, BL, C, HW))
        xhi, xlo = _hi_lo(xc)
        m = dict(common)
        m["xhi"] = xhi
        if L1_MODE == "dr":
            xc8 = np.empty((T, BL, C, 2, HW), dtype=ml_dtypes.float8_e5m2)
            xc8[:, :, :, 0, :] = _e5m2(xc, 1.0 / DR_S)
            xc8[:, :, :, 1, :] = _e5m2(xlo.astype(np.float32), DR_S)
            m["xc8"] = xc8
        elif TERMS_L1 >= 2:
            m["xlo"] = xlo
        in_maps.append(m)
    return in_maps


def kernel(inp, conv_w1, conv_w2, conv_w3, bn_w1, bn_b1, bn_w2, bn_b2,
           bn_w3, bn_b3, fc_w, fc_b):
    inp = np.asarray(inp, dtype=np.float32)
    ws = [np.asarray(w, dtype=np.float32) for w in (conv_w1, conv_w2, conv_w3)]
    bns = [(np.asarray(bn_w1, np.float32), np.asarray(bn_b1, np.float32)),
           (np.asarray(bn_w2, np.float32), np.asarray(bn_b2, np.float32)),
           (np.asarray(bn_w3, np.float32), np.asarray(bn_b3, np.float32))]
    fc_w = np.asarray(fc_w, np.float32)
    fc_b = np.asarray(fc_b, np.float32)

    nc = _get_nc()
    in_maps = make_in_maps(inp, ws, bns, fc_w, fc_b)
    res = run_bass_kernel_spmd(nc, in_maps, core_ids=list(range(NCORES)))
    out = np.concatenate(
        [r["out"].reshape(BL, 10) for r in res.results], axis=0)
    return out.astype(np.float32)

